# revision 34
# baseline (speedup 1.0000x reference)
"""Single-head attention kernel for Trainium2, SPMD over 8 NeuronCores.

Problem: x [4,4096,128], Wq/Wk/Wv [128,128] -> y [4,4096,128]
  q = x @ Wq.T ; k = x @ Wk.T ; v = x @ Wv.T
  y = softmax(q k^T / sqrt(128)) v

Sharding: 8 cores = 4 batches x 2 query-halves. Each core receives its
batch's x rotated so that its 2048 queries are rows 0..2047 (attention is
invariant to permuting the key order) -> all cores run the identical NEFF
with no dynamic offsets and no collectives.

Per-core dataflow (all attention matmuls bf16 inputs, f32 PSUM accum):
  x chunks: f32 HWDGE loads on the sync queue (all 8 upfront), DVE cast
  to bf16, PE transpose (PSUM->SBUF copies split ACT/DVE). W f32 on the
  scalar HWDGE queue in parallel; M = Wq^T Wk (bf16); uT = M^T @ xT.
  per 1024-query block, per 32 key tiles:
      S^T = xT-tile^T @ uT-block     (PE, 2x N=512 into [128k,1024] PSUM)
      A^T = exp(S^T * scale)         (ACT; kt%5==2 on DVE via Schraudolph)
      yT += x-tile^T @ A^T           (PE, [128h,1024q] PSUM accum)
      pair adds + running spine of A^T (DVE only: gpsimd elementwise
      serializes against DVE on TRN2, so it is left idle) -> one root
      covering kts 0..29
  l column j = PSUM-accum group of 3 tiny matmuls (spine/a30/a31 chunk_j
  as lhsT x ones) + per-column reciprocal; y = (w_sb_chunk^T @ wvT) *
  (1/l) via ACT Copy-with-scale / DVE tensor_scalar_mul, which also
  transposes y back to row-major for free.

Single software-pipelined instruction stream: prep pieces and each
block's epilogue are sliced into small closures and interleaved into the
next block's kt loop (the tail epilogue interleaves l/y pieces and
splits output DMAs across both HWDGE queues).
"""

import sys

sys.path.insert(0, "/opt/trn_rl_repo")

import numpy as np

import concourse.bass as bass
import concourse.mybir as mybir
from concourse import bacc
from concourse.bass_utils import run_bass_kernel_spmd
from concourse.tile import TileContext
from concourse.masks import make_identity

P = 128
N = 4096  # context length (per batch)
NQ = 2048  # queries per core
H = 128
O = 128
KT = N // P  # 32 key tiles
NC = N // 512  # 8 column chunks of 512
QBS = 1024  # query block size
QB = NQ // QBS  # 2 query blocks
SCALE = 1.0 / np.sqrt(128.0)
# Schraudolph exp-as-bf16-bits: bf16bits(exp(s*SCALE)) ~= s*ES0 + ES1
ES0 = float(128.0 * np.log2(np.e) * SCALE)
ES1 = float((127 << 7) - 8.0)

F32 = mybir.dt.float32
BF16 = mybir.dt.bfloat16

_cached_nc = None


def build_kernel():
    nc = bacc.Bacc(None, target_bir_lowering=False)

    x_d = nc.declare_dram_parameter("x", [N, H], F32, isOutput=False)
    w_d = {
        "q": nc.declare_dram_parameter("wq", [H, H], F32, isOutput=False),
        "k": nc.declare_dram_parameter("wk", [H, H], F32, isOutput=False),
        "v": nc.declare_dram_parameter("wv", [O, H], F32, isOutput=False),
    }
    out_d = nc.declare_dram_parameter("out", [NQ, O], F32, isOutput=True)

    with TileContext(nc) as tc:
        with (
            tc.tile_pool(name="const", bufs=1) as cpool,
            tc.tile_pool(name="big", bufs=1) as big,
            tc.tile_pool(name="stagea", bufs=1) as sta,
            tc.tile_pool(name="psum", bufs=2, space="PSUM") as psum,
            tc.tile_pool(name="apool", bufs=7) as apool,
            tc.tile_pool(name="tpool", bufs=4) as tpool,
            tc.tile_pool(name="spool", bufs=2) as spool,
            tc.tile_pool(name="epi", bufs=3) as epi,
        ):
            xTs = [big.tile([P, 512], BF16, name=f"xT{c}") for c in range(NC)]
            uTs = [big.tile([P, 512], BF16, name=f"uT{c}") for c in range(NQ // 512)]
            xcs = [big.tile([P, 512], BF16, name=f"xc{c}") for c in range(NC)]
            xst = [big.tile([P, 4, P], F32, name=f"xst{c}") for c in range(NC)]
            wvT = big.tile([P, P], BF16, name="wvT")
            m_sb = big.tile([P, P], BF16, name="m_sb")

            def kslice(tiles, kt):
                return tiles[kt // 4][:, (kt % 4) * P : (kt % 4 + 1) * P]

            # ---------- prep emitters ----------
            def chunk_load(c):
                rows = x_d[c * 512 : (c + 1) * 512, :]
                nc.sync.dma_start(
                    out=xst[c][:], in_=rows.rearrange("(t p) h -> p t h", p=P)
                )

            def chunk_cast(c):
                nc.vector.tensor_copy(
                    xcs[c][:], xst[c].rearrange("p t h -> p (t h)")
                )  # f32 -> bf16

            def w_load():
                # gpsimd SWDGE cast-DMAs: the Pool queue opens earliest
                # (~5.9us) and casts f32->bf16 inline, landing W ~2.5us
                # before the HWDGE+ACT-cast path would
                nc.gpsimd.dma_start(out=wqk[:, 0:P], in_=w_d["q"][:])
                nc.gpsimd.dma_start(out=wqk[:, P : 2 * P], in_=w_d["k"][:])
                nc.gpsimd.dma_start(out=wst[:], in_=w_d["v"][:])

            def emit_ident_ones():
                ident_bf_ = cpool.tile([P, P], BF16, name="ident_bf")
                make_identity(nc, ident_bf_)
                ones_bf_ = cpool.tile([P, 1], BF16, name="ones_bf")
                nc.gpsimd.memset(ones_bf_[:], 1.0)
                return ident_bf_, ones_bf_

            def pe_transpose(c, copy_eng):
                px = psum.tile([P, 512], BF16, tag="sm", name=f"px{c}")
                for t4 in range(4):
                    nc.tensor.transpose(
                        px[:, t4 * P : (t4 + 1) * P],
                        xcs[c][:, t4 * P : (t4 + 1) * P],
                        ident_bf[:],
                    )
                if copy_eng == "act":
                    nc.scalar.activation(
                        xTs[c][:], px[:], mybir.ActivationFunctionType.Copy
                    )
                else:
                    nc.vector.tensor_copy(xTs[c][:], px[:])

            def xbar_transpose(c):
                # late chunks: XBAR DMA transpose per 128x128 tile on the
                # otherwise-idle sync queue (~1.2us queue occupancy each,
                # arrivals ~10us+ ahead of their first S-matmul use)
                for t4 in range(4):
                    nc.sync.dma_start(
                        out=xTs[c][:, t4 * P : (t4 + 1) * P],
                        in_=xcs[c][:, t4 * P : (t4 + 1) * P],
                        transpose=True,
                    )

            wqk = big.tile([P, 2 * P], BF16, name="wqk")
            wst = big.tile([P, P], BF16, name="wst")

            def m_bf16():
                pm = psum.tile([P, P], F32, tag="sm", name="pm")
                nc.tensor.matmul(
                    pm[:], wqk[:, 0:P], wqk[:, P : 2 * P], start=True, stop=True
                )
                nc.scalar.activation(
                    m_sb[:], pm[:], mybir.ActivationFunctionType.Copy
                )

            def wv_transpose():
                pw = psum.tile([P, P], BF16, tag="sm", name="pw")
                nc.tensor.transpose(pw[:], wst[:], ident_bf[:])
                nc.vector.tensor_copy(wvT[:], pw[:])

            def chunk_u(c, copy_eng="act"):
                pu = psum.tile([P, 512], F32, tag="sm", name=f"pu{c}")
                nc.tensor.matmul(pu[:], m_sb[:], xTs[c][:], start=True, stop=True)
                if copy_eng == "act":
                    nc.scalar.activation(
                        uTs[c][:], pu[:], mybir.ActivationFunctionType.Copy
                    )
                else:
                    nc.vector.tensor_copy(uTs[c][:], pu[:])

            # ---------- attention emitters ----------
            a_tiles = {}

            def emit_s_exp(qb, kt):
                ps = psum.tile([P, QBS], F32, tag="ps")
                for h in range(QBS // 512):
                    nc.tensor.matmul(
                        ps[:, h * 512 : (h + 1) * 512],
                        kslice(xTs, kt),
                        uTs[qb * 2 + h][:],
                        start=True, stop=True,
                    )
                a = apool.tile([P, QBS], BF16, tag="a")
                if kt % 5 == 2:
                    # Schraudolph: affine map + f32->int16 convert writes the
                    # bf16 bit pattern of exp(s*SCALE) (~2% elementwise, which
                    # the softmax renormalization cancels to ~0.1% on y)
                    nc.vector.tensor_scalar(
                        a.bitcast(mybir.dt.int16)[:], ps[:], ES0, ES1,
                        mybir.AluOpType.mult, mybir.AluOpType.add,
                    )
                else:
                    nc.scalar.activation(
                        a[:], ps[:], mybir.ActivationFunctionType.Exp,
                        scale=float(SCALE),
                    )
                a_tiles[(qb, kt)] = a

            def new_blk(qb):
                return {
                    "qb": qb,
                    "py": psum.tile([P, QBS], F32, tag="py", bufs=1,
                                    name=f"py{qb}"),
                    "hold": None,     # first tile of a pending pair
                    "p1s": [],        # completed pair sums awaiting spine
                    "spine": None,    # running spine sum (pairs 0..i)
                    "nspine": 0,
                    "a3031": [],      # a tiles for kt30/31 (no tree)
                }

            def emit_spine(b):
                p1 = b["p1s"].pop(0)
                if b["spine"] is None:
                    b["spine"] = p1
                else:
                    sp = spool.tile([P, QBS], BF16, tag="sp")
                    nc.vector.tensor_tensor(
                        sp[:], b["spine"][:], p1[:], mybir.AluOpType.add
                    )
                    b["spine"] = sp
                b["nspine"] += 1

            def emit_kt(b, kt, gp_lo, gp_hi):
                qb = b["qb"]
                a = a_tiles.pop((qb, kt))
                for h in range(QBS // 512):
                    nc.tensor.matmul(
                        b["py"][:, h * 512 : (h + 1) * 512],
                        kslice(xcs, kt),
                        a[:, h * 512 : (h + 1) * 512],
                        start=(kt == 0), stop=(kt == KT - 1),
                    )
                if b["hold"] is None:
                    b["hold"] = a
                    return
                # pair add (L1) for pair i = (kt-1)//2 at odd kt 1..31
                i = (kt - 1) // 2
                p1 = tpool.tile([P, QBS], BF16, tag="t1")
                eng = nc.gpsimd if gp_lo <= i <= gp_hi else nc.vector
                eng.tensor_tensor(p1[:], b["hold"][:], a[:], mybir.AluOpType.add)
                b["hold"] = None
                b["p1s"].append(p1)
                # spine add, lagged 2 pairs behind L1 production; drain the
                # remainder across kts 29/31 so the root closes with kt31
                if len(b["p1s"]) >= 3 or kt == 29:
                    emit_spine(b)
                if kt == 29 and b["p1s"]:
                    emit_spine(b)
                if kt == 31:
                    while b["p1s"]:
                        emit_spine(b)

            # ---------- epilogue ----------
            def epilogue_pieces(b, last=False):
                qb = b["qb"]
                st = {}

                def p_w():
                    # py -> SBUF bf16. Mid-kernel: halves split ACT/DVE; at
                    # the tail the DVE is busy closing the spine, so both
                    # halves go on ACT.
                    w_sb = epi.tile([P, QBS], BF16, tag="w_sb", name=f"w{qb}")
                    nc.scalar.activation(
                        w_sb[:, 0:512], b["py"][:, 0:512],
                        mybir.ActivationFunctionType.Copy,
                    )
                    if last:
                        nc.scalar.activation(
                            w_sb[:, 512:QBS], b["py"][:, 512:QBS],
                            mybir.ActivationFunctionType.Copy,
                        )
                    else:
                        nc.vector.tensor_copy(
                            w_sb[:, 512:QBS], b["py"][:, 512:QBS]
                        )
                    st["w_sb"] = w_sb

                def p_lj(j):
                    # l column j = root_chunk_j^T @ ones (single matmul)
                    if j == 0:
                        st["pl"] = psum.tile([P, 8], F32, tag="sm",
                                             name=f"pl{qb}")
                        st["lcol"] = epi.tile([P, 8], F32, tag="lcol",
                                              name=f"lc{qb}")
                    nc.tensor.matmul(
                        st["pl"][:, j : j + 1],
                        b["spine"][:, j * P : (j + 1) * P],
                        ones_bf[:],
                        start=True, stop=True,
                    )
                    if last:
                        # tail: per-column recip so p_j(j) can chase p_lj(j)
                        nc.vector.reciprocal(
                            st["lcol"][:, j : j + 1], st["pl"][:, j : j + 1]
                        )
                    elif j == 7:
                        nc.vector.reciprocal(st["lcol"][:], st["pl"][:])

                def p_j(j):
                    g = j // 4
                    if j % 4 == 0:
                        st[f"yout{g}"] = epi.tile(
                            [P, 4, P], F32, tag="yout", name=f"yout{qb}_{g}"
                        )
                    psm_y = psum.tile([P, P], F32, tag="ps" if last else "sm")
                    nc.tensor.matmul(
                        psm_y[:], st["w_sb"][:, j * P : (j + 1) * P], wvT[:],
                        start=True, stop=True,
                    )
                    lc = st["lcol"][:, j : j + 1]
                    if j % 2 == 0:
                        nc.scalar.activation(
                            st[f"yout{g}"][:, j % 4, :], psm_y[:],
                            mybir.ActivationFunctionType.Copy, scale=lc,
                        )
                    else:
                        nc.vector.tensor_scalar_mul(
                            st[f"yout{g}"][:, j % 4, :], psm_y[:], lc
                        )
                    if not last:
                        if j % 4 == 3:
                            r0 = qb * QBS + g * 512
                            nc.sync.dma_start(
                                out=out_d[r0 : r0 + 512, :].rearrange(
                                    "(t p) h -> p t h", p=P
                                ),
                                in_=st[f"yout{g}"][:],
                            )
                    else:
                        if j % 2 == 1:
                            # tail: finer DMAs split across both HWDGE queues
                            r0 = qb * QBS + (j - 1) * P
                            eng = nc.sync if (j // 2) % 2 == 0 else nc.scalar
                            eng.dma_start(
                                out=out_d[r0 : r0 + 256, :].rearrange(
                                    "(t p) h -> p t h", p=P
                                ),
                                in_=st[f"yout{g}"][:, (j - 1) % 4 : (j - 1) % 4 + 2, :],
                            )

                if last:
                    # psm_y lives on "ps" here, so pl ("sm") is safe: the
                    # l/y pieces can interleave to shorten the tail
                    pieces = [lambda: p_lj(0), lambda: p_lj(1)]
                    nxt_l = 2
                    for j in range(8):
                        pieces.append(lambda j=j: p_j(j))
                        if nxt_l < 8:
                            pieces.append(lambda k=nxt_l: p_lj(k))
                            nxt_l += 1
                else:
                    # all l-pieces before y-pieces: pl ("sm" ring) must be
                    # dead before the second "sm" psm_y alloc reuses its slot
                    pieces = [lambda j=j: p_lj(j) for j in range(8)]
                    pieces += [lambda j=j: p_j(j) for j in range(8)]
                return p_w, pieces

            # ---------- emission schedule ----------
            w_load()
            chunk_load(0)
            chunk_load(1)
            ident_bf, ones_bf = emit_ident_ones()
            for c in range(2, NC):
                chunk_load(c)
            m_bf16()
            chunk_cast(0)
            chunk_cast(1)
            pe_transpose(0, "vec")
            pe_transpose(1, "vec")
            chunk_u(0, "act")
            chunk_u(1, "vec")

            prep_todo = [
                lambda: chunk_cast(2),
                lambda: pe_transpose(2, "act"),
                lambda: chunk_cast(3),
                lambda: pe_transpose(3, "vec"),
                lambda: chunk_u(2),
                lambda: chunk_cast(4),
                lambda: pe_transpose(4, "act"),
                lambda: chunk_u(3),
                lambda: chunk_cast(5),
                lambda: pe_transpose(5, "vec"),
                lambda: chunk_cast(6),
                lambda: pe_transpose(6, "act"),
                lambda: chunk_cast(7),
                lambda: pe_transpose(7, "vec"),
                wv_transpose,
            ]

            blk = None
            todo = None
            for qb in range(QB):
                if qb == 0:
                    emit_s_exp(0, 0)
                    emit_s_exp(0, 1)
                    todo = prep_todo
                    gp_lo, gp_hi = 1, 0  # gpsimd serializes with DVE: unused
                else:
                    p_w, todo = epilogue_pieces(blk)
                    p_w()                 # before next block's kt0 (py WAR)
                    gp_lo, gp_hi = 1, 0
                blk = new_blk(qb)

                for kt in range(KT):
                    if kt + 2 < KT:
                        emit_s_exp(qb, kt + 2)
                    elif qb + 1 < QB:
                        emit_s_exp(qb + 1, kt + 2 - KT)
                    emit_kt(blk, kt, gp_lo, gp_hi)
                    if todo:
                        todo.pop(0)()
                while todo:
                    todo.pop(0)()

            p_w, todo = epilogue_pieces(blk, last=True)
            p_w()
            for piece in todo:
                piece()

    nc.compile()
    return nc


def _run(x, Wq, Wk, Wv, **spmd_kwargs):
    global _cached_nc
    if _cached_nc is None:
        _cached_nc = build_kernel()
    nc = _cached_nc

    x = np.asarray(x, dtype=np.float32)
    Wq = np.ascontiguousarray(np.asarray(Wq, dtype=np.float32))
    Wk = np.ascontiguousarray(np.asarray(Wk, dtype=np.float32))
    Wv = np.ascontiguousarray(np.asarray(Wv, dtype=np.float32))

    B = x.shape[0]
    in_maps = []
    for core in range(8):
        b, half = core // 2, core % 2
        xb = x[b]
        if half:
            xb = np.roll(xb, -NQ, axis=0)  # queries -> rows 0..NQ-1
        in_maps.append(
            {"x": np.ascontiguousarray(xb), "wq": Wq, "wk": Wk, "wv": Wv}
        )

    res = run_bass_kernel_spmd(nc, in_maps, core_ids=list(range(8)), **spmd_kwargs)

    y = np.empty((B, N, O), dtype=np.float32)
    for core in range(8):
        b, half = core // 2, core % 2
        y[b, half * NQ : (half + 1) * NQ] = res.results[core]["out"]
    return y, res


def kernel(x, Wq, Wk, Wv):
    y, _ = _run(x, Wq, Wk, Wv)
    return y


if __name__ == "__main__":
    rng = np.random.default_rng(0)
    x = rng.standard_normal((4, N, H), dtype=np.float32)
    Wq = rng.standard_normal((H, H), dtype=np.float32) / np.sqrt(H)
    Wk = rng.standard_normal((H, H), dtype=np.float32) / np.sqrt(H)
    Wv = rng.standard_normal((O, H), dtype=np.float32) / np.sqrt(H)
    y = kernel(x=x, Wq=Wq, Wk=Wk, Wv=Wv)
    print("kernel output", y.shape, y.dtype)


# revision 35
# speedup vs baseline: 1.0038x; 1.0038x over previous
"""Single-head attention kernel for Trainium2, SPMD over 8 NeuronCores.

Problem: x [4,4096,128], Wq/Wk/Wv [128,128] -> y [4,4096,128]
  q = x @ Wq.T ; k = x @ Wk.T ; v = x @ Wv.T
  y = softmax(q k^T / sqrt(128)) v

Sharding: 8 cores = 4 batches x 2 query-halves. Each core receives its
batch's x rotated so that its 2048 queries are rows 0..2047 (attention is
invariant to permuting the key order) -> all cores run the identical NEFF
with no dynamic offsets and no collectives.

Per-core dataflow (all attention matmuls bf16 inputs, f32 PSUM accum):
  x chunks: f32 HWDGE loads on the sync queue (all 8 upfront), DVE cast
  to bf16, PE transpose (PSUM->SBUF copies split ACT/DVE). W f32 on the
  scalar HWDGE queue in parallel; M = Wq^T Wk (bf16); uT = M^T @ xT.
  per 1024-query block, per 32 key tiles:
      S^T = xT-tile^T @ uT-block     (PE, 2x N=512 into [128k,1024] PSUM)
      A^T = exp(S^T * scale)         (ACT; kt%5==2 on DVE via Schraudolph)
      yT += x-tile^T @ A^T           (PE, [128h,1024q] PSUM accum)
      pair adds + running spine of A^T (DVE only: gpsimd elementwise
      serializes against DVE on TRN2, so it is left idle) -> one root
      covering kts 0..29
  l column j = PSUM-accum group of 3 tiny matmuls (spine/a30/a31 chunk_j
  as lhsT x ones) + per-column reciprocal; y = (w_sb_chunk^T @ wvT) *
  (1/l) via ACT Copy-with-scale / DVE tensor_scalar_mul, which also
  transposes y back to row-major for free.

Single software-pipelined instruction stream: prep pieces and each
block's epilogue are sliced into small closures and interleaved into the
next block's kt loop (the tail epilogue interleaves l/y pieces and
splits output DMAs across both HWDGE queues).
"""

import sys

sys.path.insert(0, "/opt/trn_rl_repo")

import numpy as np

import concourse.bass as bass
import concourse.mybir as mybir
from concourse import bacc
from concourse.bass_utils import run_bass_kernel_spmd
from concourse.tile import TileContext
from concourse.masks import make_identity

P = 128
N = 4096  # context length (per batch)
NQ = 2048  # queries per core
H = 128
O = 128
KT = N // P  # 32 key tiles
NC = N // 512  # 8 column chunks of 512
QBS = 1024  # query block size
QB = NQ // QBS  # 2 query blocks
SCALE = 1.0 / np.sqrt(128.0)
# Schraudolph exp-as-bf16-bits: bf16bits(exp(s*SCALE)) ~= s*ES0 + ES1
ES0 = float(128.0 * np.log2(np.e) * SCALE)
ES1 = float((127 << 7) - 8.0)

F32 = mybir.dt.float32
BF16 = mybir.dt.bfloat16

_cached_nc = None


def build_kernel():
    nc = bacc.Bacc(None, target_bir_lowering=False)

    x_d = nc.declare_dram_parameter("x", [N, H], F32, isOutput=False)
    w_d = {
        "q": nc.declare_dram_parameter("wq", [H, H], F32, isOutput=False),
        "k": nc.declare_dram_parameter("wk", [H, H], F32, isOutput=False),
        "v": nc.declare_dram_parameter("wv", [O, H], F32, isOutput=False),
    }
    out_d = nc.declare_dram_parameter("out", [NQ, O], F32, isOutput=True)

    with TileContext(nc) as tc:
        with (
            tc.tile_pool(name="const", bufs=1) as cpool,
            tc.tile_pool(name="big", bufs=1) as big,
            tc.tile_pool(name="stagea", bufs=1) as sta,
            tc.tile_pool(name="psum", bufs=2, space="PSUM") as psum,
            tc.tile_pool(name="apool", bufs=7) as apool,
            tc.tile_pool(name="tpool", bufs=4) as tpool,
            tc.tile_pool(name="spool", bufs=2) as spool,
            tc.tile_pool(name="epi", bufs=3) as epi,
        ):
            xTs = [big.tile([P, 512], BF16, name=f"xT{c}") for c in range(NC)]
            uTs = [big.tile([P, 512], BF16, name=f"uT{c}") for c in range(NQ // 512)]
            xcs = [big.tile([P, 512], BF16, name=f"xc{c}") for c in range(NC)]
            xst = [big.tile([P, 4, P], F32, name=f"xst{c}") for c in range(NC)]
            wvT = big.tile([P, P], BF16, name="wvT")
            m_sb = big.tile([P, P], BF16, name="m_sb")
            wf = sta.tile([P, 3 * P], F32, name="wf")

            def kslice(tiles, kt):
                return tiles[kt // 4][:, (kt % 4) * P : (kt % 4 + 1) * P]

            # ---------- prep emitters ----------
            def chunk_load(c):
                rows = x_d[c * 512 : (c + 1) * 512, :]
                nc.sync.dma_start(
                    out=xst[c][:], in_=rows.rearrange("(t p) h -> p t h", p=P)
                )

            def chunk_cast(c):
                nc.vector.tensor_copy(
                    xcs[c][:], xst[c].rearrange("p t h -> p (t h)")
                )  # f32 -> bf16

            def w_load():
                # W f32 on the scalar HWDGE queue (parallel with x loads)
                for wi, name in enumerate(("q", "k", "v")):
                    nc.scalar.dma_start(
                        out=wf[:, wi * P : (wi + 1) * P], in_=w_d[name][:]
                    )

            def emit_ident_ones():
                ident_bf_ = cpool.tile([P, P], BF16, name="ident_bf")
                make_identity(nc, ident_bf_)
                ones_bf_ = cpool.tile([P, 1], BF16, name="ones_bf")
                nc.gpsimd.memset(ones_bf_[:], 1.0)
                return ident_bf_, ones_bf_

            def pe_transpose(c, copy_eng):
                px = psum.tile([P, 512], BF16, tag="sm", name=f"px{c}")
                for t4 in range(4):
                    nc.tensor.transpose(
                        px[:, t4 * P : (t4 + 1) * P],
                        xcs[c][:, t4 * P : (t4 + 1) * P],
                        ident_bf[:],
                    )
                if copy_eng == "act":
                    nc.scalar.activation(
                        xTs[c][:], px[:], mybir.ActivationFunctionType.Copy
                    )
                else:
                    nc.vector.tensor_copy(xTs[c][:], px[:])

            def xbar_transpose(c):
                # late chunks: XBAR DMA transpose per 128x128 tile on the
                # otherwise-idle sync queue (~1.2us queue occupancy each,
                # arrivals ~10us+ ahead of their first S-matmul use)
                for t4 in range(4):
                    nc.sync.dma_start(
                        out=xTs[c][:, t4 * P : (t4 + 1) * P],
                        in_=xcs[c][:, t4 * P : (t4 + 1) * P],
                        transpose=True,
                    )

            wqk = big.tile([P, 2 * P], BF16, name="wqk")

            def w_casts():
                # single ACT op casting Wq|Wk together
                nc.scalar.activation(
                    wqk[:], wf[:, 0 : 2 * P],
                    mybir.ActivationFunctionType.Copy,
                )

            def m_bf16():
                pm = psum.tile([P, P], F32, tag="sm", name="pm")
                nc.tensor.matmul(
                    pm[:], wqk[:, 0:P], wqk[:, P : 2 * P], start=True, stop=True
                )
                nc.scalar.activation(
                    m_sb[:], pm[:], mybir.ActivationFunctionType.Copy
                )

            wst_holder = {}

            def wv_cast():
                wst = sta.tile([P, P], BF16, name="wst")
                nc.scalar.activation(
                    wst[:], wf[:, 2 * P : 3 * P],
                    mybir.ActivationFunctionType.Copy,
                )
                wst_holder["wst"] = wst

            def wv_transpose():
                pw = psum.tile([P, P], BF16, tag="sm", name="pw")
                nc.tensor.transpose(pw[:], wst_holder["wst"][:], ident_bf[:])
                nc.vector.tensor_copy(wvT[:], pw[:])

            def chunk_u(c, copy_eng="act"):
                pu = psum.tile([P, 512], F32, tag="sm", name=f"pu{c}")
                nc.tensor.matmul(pu[:], m_sb[:], xTs[c][:], start=True, stop=True)
                if copy_eng == "act":
                    nc.scalar.activation(
                        uTs[c][:], pu[:], mybir.ActivationFunctionType.Copy
                    )
                else:
                    nc.vector.tensor_copy(uTs[c][:], pu[:])

            # ---------- attention emitters ----------
            a_tiles = {}

            def emit_s_exp(qb, kt):
                ps = psum.tile([P, QBS], F32, tag="ps")
                for h in range(QBS // 512):
                    nc.tensor.matmul(
                        ps[:, h * 512 : (h + 1) * 512],
                        kslice(xTs, kt),
                        uTs[qb * 2 + h][:],
                        start=True, stop=True,
                    )
                a = apool.tile([P, QBS], BF16, tag="a")
                if kt % 5 == 2:
                    # Schraudolph: affine map + f32->int16 convert writes the
                    # bf16 bit pattern of exp(s*SCALE) (~2% elementwise, which
                    # the softmax renormalization cancels to ~0.1% on y)
                    nc.vector.tensor_scalar(
                        a.bitcast(mybir.dt.int16)[:], ps[:], ES0, ES1,
                        mybir.AluOpType.mult, mybir.AluOpType.add,
                    )
                else:
                    nc.scalar.activation(
                        a[:], ps[:], mybir.ActivationFunctionType.Exp,
                        scale=float(SCALE),
                    )
                a_tiles[(qb, kt)] = a

            def new_blk(qb):
                return {
                    "qb": qb,
                    "py": psum.tile([P, QBS], F32, tag="py", bufs=1,
                                    name=f"py{qb}"),
                    "hold": None,     # first tile of a pending pair
                    "p1s": [],        # completed pair sums awaiting spine
                    "spine": None,    # running spine sum (pairs 0..i)
                    "nspine": 0,
                    "a3031": [],      # a tiles for kt30/31 (no tree)
                }

            def emit_spine(b):
                p1 = b["p1s"].pop(0)
                if b["spine"] is None:
                    b["spine"] = p1
                else:
                    sp = spool.tile([P, QBS], BF16, tag="sp")
                    nc.vector.tensor_tensor(
                        sp[:], b["spine"][:], p1[:], mybir.AluOpType.add
                    )
                    b["spine"] = sp
                b["nspine"] += 1

            def emit_kt(b, kt, gp_lo, gp_hi):
                qb = b["qb"]
                a = a_tiles.pop((qb, kt))
                for h in range(QBS // 512):
                    nc.tensor.matmul(
                        b["py"][:, h * 512 : (h + 1) * 512],
                        kslice(xcs, kt),
                        a[:, h * 512 : (h + 1) * 512],
                        start=(kt == 0), stop=(kt == KT - 1),
                    )
                if b["hold"] is None:
                    b["hold"] = a
                    return
                # pair add (L1) for pair i = (kt-1)//2 at odd kt 1..31
                i = (kt - 1) // 2
                p1 = tpool.tile([P, QBS], BF16, tag="t1")
                eng = nc.gpsimd if gp_lo <= i <= gp_hi else nc.vector
                eng.tensor_tensor(p1[:], b["hold"][:], a[:], mybir.AluOpType.add)
                b["hold"] = None
                b["p1s"].append(p1)
                # spine add, lagged 2 pairs behind L1 production; drain the
                # remainder across kts 29/31 so the root closes with kt31
                if len(b["p1s"]) >= 3 or kt == 29:
                    emit_spine(b)
                if kt == 29 and b["p1s"]:
                    emit_spine(b)
                if kt == 31:
                    while b["p1s"]:
                        emit_spine(b)

            # ---------- epilogue ----------
            def epilogue_pieces(b, last=False):
                qb = b["qb"]
                st = {}

                def p_w():
                    # py -> SBUF bf16. Mid-kernel: halves split ACT/DVE; at
                    # the tail the DVE is busy closing the spine, so both
                    # halves go on ACT.
                    w_sb = epi.tile([P, QBS], BF16, tag="w_sb", name=f"w{qb}")
                    nc.scalar.activation(
                        w_sb[:, 0:512], b["py"][:, 0:512],
                        mybir.ActivationFunctionType.Copy,
                    )
                    if last:
                        nc.scalar.activation(
                            w_sb[:, 512:QBS], b["py"][:, 512:QBS],
                            mybir.ActivationFunctionType.Copy,
                        )
                    else:
                        nc.vector.tensor_copy(
                            w_sb[:, 512:QBS], b["py"][:, 512:QBS]
                        )
                    st["w_sb"] = w_sb

                def p_lj(j):
                    # l column j = root_chunk_j^T @ ones (single matmul)
                    if j == 0:
                        st["pl"] = psum.tile([P, 8], F32, tag="sm",
                                             name=f"pl{qb}")
                        st["lcol"] = epi.tile([P, 8], F32, tag="lcol",
                                              name=f"lc{qb}")
                    nc.tensor.matmul(
                        st["pl"][:, j : j + 1],
                        b["spine"][:, j * P : (j + 1) * P],
                        ones_bf[:],
                        start=True, stop=True,
                    )
                    if last:
                        # tail: per-column recip so p_j(j) can chase p_lj(j)
                        nc.vector.reciprocal(
                            st["lcol"][:, j : j + 1], st["pl"][:, j : j + 1]
                        )
                    elif j == 7:
                        nc.vector.reciprocal(st["lcol"][:], st["pl"][:])

                def p_j(j):
                    g = j // 4
                    if j % 4 == 0:
                        st[f"yout{g}"] = epi.tile(
                            [P, 4, P], F32, tag="yout", name=f"yout{qb}_{g}"
                        )
                    psm_y = psum.tile([P, P], F32, tag="ps" if last else "sm")
                    nc.tensor.matmul(
                        psm_y[:], st["w_sb"][:, j * P : (j + 1) * P], wvT[:],
                        start=True, stop=True,
                    )
                    lc = st["lcol"][:, j : j + 1]
                    if j % 2 == 0:
                        nc.scalar.activation(
                            st[f"yout{g}"][:, j % 4, :], psm_y[:],
                            mybir.ActivationFunctionType.Copy, scale=lc,
                        )
                    else:
                        nc.vector.tensor_scalar_mul(
                            st[f"yout{g}"][:, j % 4, :], psm_y[:], lc
                        )
                    if not last:
                        if j % 4 == 3:
                            r0 = qb * QBS + g * 512
                            nc.sync.dma_start(
                                out=out_d[r0 : r0 + 512, :].rearrange(
                                    "(t p) h -> p t h", p=P
                                ),
                                in_=st[f"yout{g}"][:],
                            )
                    else:
                        if j % 2 == 1:
                            # tail: finer DMAs split across both HWDGE queues
                            r0 = qb * QBS + (j - 1) * P
                            eng = nc.sync if (j // 2) % 2 == 0 else nc.scalar
                            eng.dma_start(
                                out=out_d[r0 : r0 + 256, :].rearrange(
                                    "(t p) h -> p t h", p=P
                                ),
                                in_=st[f"yout{g}"][:, (j - 1) % 4 : (j - 1) % 4 + 2, :],
                            )

                if last:
                    # psm_y lives on "ps" here, so pl ("sm") is safe: the
                    # l/y pieces can interleave to shorten the tail
                    pieces = [lambda: p_lj(0), lambda: p_lj(1)]
                    nxt_l = 2
                    for j in range(8):
                        pieces.append(lambda j=j: p_j(j))
                        if nxt_l < 8:
                            pieces.append(lambda k=nxt_l: p_lj(k))
                            nxt_l += 1
                else:
                    # all l-pieces before y-pieces: pl ("sm" ring) must be
                    # dead before the second "sm" psm_y alloc reuses its slot
                    pieces = [lambda j=j: p_lj(j) for j in range(8)]
                    pieces += [lambda j=j: p_j(j) for j in range(8)]
                return p_w, pieces

            # ---------- emission schedule ----------
            chunk_load(0)
            chunk_load(1)
            w_load()
            ident_bf, ones_bf = emit_ident_ones()
            for c in range(2, NC):
                chunk_load(c)
            w_casts()
            chunk_cast(0)
            chunk_cast(1)
            pe_transpose(0, "vec")
            pe_transpose(1, "vec")
            m_bf16()
            chunk_u(0, "act")
            chunk_u(1, "vec")

            prep_todo = [
                lambda: chunk_cast(2),
                lambda: pe_transpose(2, "act"),
                lambda: chunk_cast(3),
                lambda: pe_transpose(3, "vec"),
                lambda: chunk_u(2),
                lambda: chunk_cast(4),
                lambda: pe_transpose(4, "act"),
                lambda: chunk_u(3),
                lambda: chunk_cast(5),
                lambda: pe_transpose(5, "vec"),
                lambda: chunk_cast(6),
                lambda: pe_transpose(6, "act"),
                lambda: chunk_cast(7),
                lambda: pe_transpose(7, "vec"),
                wv_cast,
                wv_transpose,
            ]

            blk = None
            todo = None
            for qb in range(QB):
                if qb == 0:
                    emit_s_exp(0, 0)
                    emit_s_exp(0, 1)
                    todo = prep_todo
                    gp_lo, gp_hi = 1, 0  # gpsimd serializes with DVE: unused
                else:
                    p_w, todo = epilogue_pieces(blk)
                    p_w()                 # before next block's kt0 (py WAR)
                    gp_lo, gp_hi = 1, 0
                blk = new_blk(qb)

                for kt in range(KT):
                    if kt + 2 < KT:
                        emit_s_exp(qb, kt + 2)
                    elif qb + 1 < QB:
                        emit_s_exp(qb + 1, kt + 2 - KT)
                    emit_kt(blk, kt, gp_lo, gp_hi)
                    if todo:
                        todo.pop(0)()
                while todo:
                    todo.pop(0)()

            p_w, todo = epilogue_pieces(blk, last=True)
            p_w()
            for piece in todo:
                piece()

    nc.compile()
    return nc


def _run(x, Wq, Wk, Wv, **spmd_kwargs):
    global _cached_nc
    if _cached_nc is None:
        _cached_nc = build_kernel()
    nc = _cached_nc

    x = np.asarray(x, dtype=np.float32)
    Wq = np.ascontiguousarray(np.asarray(Wq, dtype=np.float32))
    Wk = np.ascontiguousarray(np.asarray(Wk, dtype=np.float32))
    Wv = np.ascontiguousarray(np.asarray(Wv, dtype=np.float32))

    B = x.shape[0]
    in_maps = []
    for core in range(8):
        b, half = core // 2, core % 2
        xb = x[b]
        if half:
            xb = np.roll(xb, -NQ, axis=0)  # queries -> rows 0..NQ-1
        in_maps.append(
            {"x": np.ascontiguousarray(xb), "wq": Wq, "wk": Wk, "wv": Wv}
        )

    res = run_bass_kernel_spmd(nc, in_maps, core_ids=list(range(8)), **spmd_kwargs)

    y = np.empty((B, N, O), dtype=np.float32)
    for core in range(8):
        b, half = core // 2, core % 2
        y[b, half * NQ : (half + 1) * NQ] = res.results[core]["out"]
    return y, res


def kernel(x, Wq, Wk, Wv):
    y, _ = _run(x, Wq, Wk, Wv)
    return y


if __name__ == "__main__":
    rng = np.random.default_rng(0)
    x = rng.standard_normal((4, N, H), dtype=np.float32)
    Wq = rng.standard_normal((H, H), dtype=np.float32) / np.sqrt(H)
    Wk = rng.standard_normal((H, H), dtype=np.float32) / np.sqrt(H)
    Wv = rng.standard_normal((O, H), dtype=np.float32) / np.sqrt(H)
    y = kernel(x=x, Wq=Wq, Wk=Wk, Wv=Wv)
    print("kernel output", y.shape, y.dtype)
